# revision 1
# baseline (speedup 1.0000x reference)
"""DCNv2 Trainium2 Bass kernel, v2.

Problem: x[8,64,64,128], offset[8,64,64,18], modulation[8,64,64,9],
conv_kernel[3,3,128,256], conv_bias[256] -> out[8,64,64,256].
Data-parallel over batch B=8, one batch per NeuronCore.

v2 changes vs v1 baseline:
  - img2 stored fp8 (e3m4): gather traffic halved (512B/sample elems).
    Host-measured rms rel err ~1.35% (gate 2e-2).
  - img2 built directly from x via 2 casted SWDGE DMAs + border zero DMAs
    (no bf16 img intermediate).
  - 4 SWDGE queues; gathers round-robin so transfers overlap.
  - diag-weight build via uint32 bitwise-AND against a persistent diagonal
    mask (halves DVE rows vs bf16 tensor_mul; exact bit math).
  - idx pipeline unchanged (16-partition layout feeding Q7 gather).
"""

import os
import sys

import numpy as np

sys.path.insert(0, "/opt/trn_rl_repo")

import concourse.bass as bass  # noqa: E402
import concourse.mybir as mybir  # noqa: E402
from concourse.tile import TileContext  # noqa: E402

F32 = mybir.dt.float32
BF16 = mybir.dt.bfloat16
FP8 = mybir.dt.float8e3
U32 = mybir.dt.uint32
I16 = mybir.dt.int16

H = W = 64
C = 128
F = 256
NK = 9
NPIX = H * W          # 4096 pixels per batch
IW = 68               # padded image width (66 needed + 2 slack)
NT = NPIX // 128      # 32 pixel tiles
NTK = NT * NK
MAGIC = 1.5 * 2.0**23  # fp32 round-to-int trick
CLIP_MAX = 65.0
NQ = 4                 # SWDGE queues

KY = np.array([k // 3 - 1 for k in range(9)], np.float32)
KX = np.array([k % 3 - 1 for k in range(9)], np.float32)


def _host_consts():
    p = np.arange(128)
    t = np.arange(NT)
    h = 2 * t[None, :, None] + (p[:, None, None] // 64)
    w = (p[:, None, None] % 64)
    byp = (h + 1 + KY[None, None, :]).astype(np.float32).reshape(128, NTK)
    bxp = (np.broadcast_to(w, (128, NT, 1)) + 1 + KX[None, None, :]).astype(
        np.float32
    ).reshape(128, NTK)

    # 16-part layout for idx: pixel P = t*128 + g*16 + q
    q = np.arange(16)[:, None, None, None]
    tt = t[None, :, None, None]
    kk = np.arange(NK)[None, None, :, None]
    g = np.arange(8)[None, None, None, :]
    P = tt * 128 + g * 16 + q
    h16 = P // 64
    w16 = P % 64
    by16 = (h16 + 1 + KY[kk]).astype(np.float32).reshape(16, NTK * 8)
    bx16 = (w16 + 1 + KX[kk]).astype(np.float32).reshape(16, NTK * 8)

    # diagonal all-ones-bits mask, bf16 container
    mask = np.where(np.eye(128, dtype=bool), np.uint16(0xFFFF), np.uint16(0))
    import ml_dtypes

    return {
        "byp": byp,
        "bxp": bxp,
        "by16": by16,
        "bx16": bx16,
        "mask": mask.view(ml_dtypes.bfloat16),
    }


def build_nc():
    from concourse.bacc import Bacc

    nc = Bacc(num_swdge_queues=NQ)

    x = nc.dram_tensor("x", [NPIX, C], F32, kind="ExternalInput")
    off = nc.dram_tensor("off", [NPIX, 2 * NK], F32, kind="ExternalInput")
    mod = nc.dram_tensor("mod", [NPIX, NK], F32, kind="ExternalInput")
    ck = nc.dram_tensor("ck", [NK, C, F], F32, kind="ExternalInput")
    byp_d = nc.dram_tensor("byp", [128, NTK], F32, kind="ExternalInput")
    bxp_d = nc.dram_tensor("bxp", [128, NTK], F32, kind="ExternalInput")
    by16_d = nc.dram_tensor("by16", [16, NTK * 8], F32, kind="ExternalInput")
    bx16_d = nc.dram_tensor("bx16", [16, NTK * 8], F32, kind="ExternalInput")
    mask_d = nc.dram_tensor("mask", [128, 128], BF16, kind="ExternalInput")
    out = nc.dram_tensor("out", [NPIX, F], F32, kind="ExternalOutput")

    # img2[j*IW+xx] = (img[j-1, xx], img[j, xx]) fp8 row pairs; j = ye+1
    img2 = nc.dram_tensor("img2", [67 * IW, 2 * C], FP8, kind="Internal")
    zs = nc.dram_tensor("zs", [35840], FP8, kind="Internal")
    idxd = nc.dram_tensor("idxd", [4 * 16 * 576], I16, kind="Internal")

    with TileContext(nc) as tc:
        _body(tc, x, off, mod, ck, byp_d, bxp_d, by16_d, bx16_d, mask_d, out,
              img2, zs, idxd)
    nc.finalize()
    return nc


def _body(tc, x, off, mod, ck, byp_d, bxp_d, by16_d, bx16_d, mask_d, out,
          img2, zs, idxd):
    import contextlib

    nc = tc.nc
    alu = mybir.AluOpType
    act_copy = mybir.ActivationFunctionType.Copy
    ctx = contextlib.ExitStack()
    cpool = ctx.enter_context(tc.tile_pool(name="consts", bufs=1))
    spool = ctx.enter_context(tc.tile_pool(name="setup", bufs=1))
    wpool = ctx.enter_context(tc.tile_pool(name="work", bufs=9))
    s6pool = ctx.enter_context(tc.tile_pool(name="st16", bufs=6))
    gpool = ctx.enter_context(tc.tile_pool(name="gather", bufs=5))
    dgpool = ctx.enter_context(tc.tile_pool(name="diag", bufs=2))
    opool = ctx.enter_context(tc.tile_pool(name="outs", bufs=2))
    ppool = ctx.enter_context(tc.tile_pool(name="psum", bufs=4, space="PSUM"))
    p2pool = ctx.enter_context(tc.tile_pool(name="psum2", bufs=2, space="PSUM"))

    # ---------------- persistent tiles ----------------
    byp = cpool.tile([128, NTK], F32, name="bypt")
    bxp = cpool.tile([128, NTK], F32, name="bxpt")
    mask = cpool.tile([128, 128], BF16, name="maskt")
    kmat = cpool.tile([128, NK * F], BF16, name="kmatt")
    w4dup = cpool.tile([128, NTK * 4 * 2], BF16, name="w4dup")
    idx = cpool.tile([128, NT * 72], I16, name="idx")
    feats = cpool.tile([128, 3 * 192 * 64], BF16, name="featsbuf")
    zt = cpool.tile([128, 280], FP8, name="zt")

    # ---------------- Stage A: padded fp8 image ----------------------
    # zeros staging: memset SBUF, park in DRAM for border fills
    nc.vector.memset(zt[:], 0.0)
    nc.sync.dma_start(
        out=zs[:].rearrange("(p n) -> p n", n=280), in_=zt[:]
    )

    i2v = img2[:].rearrange("(j xx) (s c) -> j xx s c", xx=IW, s=2)
    # interior: slot0 <- x rows (j-1 in 1..64), slot1 <- x rows (j in 1..64)
    xv = x[:].rearrange("(h w) c -> h w c", w=W)
    nc.gpsimd.dma_start(out=i2v[2:66, 1:65, 0, :], in_=xv)
    nc.gpsimd.dma_start(out=i2v[1:65, 1:65, 1, :], in_=xv)
    # borders (zero): full rows j=0 (both slots), j=65 slot1 + j=66 both,
    # j=1 slot0, plus cols xx in {0,65,66,67} for all j.
    nc.scalar.dma_start(out=img2[0:IW, :], in_=zs[0 : IW * 256])          # j=0
    nc.scalar.dma_start(out=i2v[1, :, 0, :], in_=zs[0 : IW * 128])        # j=1 s0
    nc.scalar.dma_start(out=i2v[65, :, 1, :], in_=zs[0 : IW * 128])       # j=65 s1
    nc.scalar.dma_start(out=img2[66 * IW : 67 * IW, :], in_=zs[0 : IW * 256])
    zcol = bass.AP(zs[:].tensor, 0, [[0, 67], [1, 256]])
    nc.scalar.dma_start(
        out=bass.AP(img2[:].tensor, 0, [[IW * 256, 67], [1, 256]]), in_=zcol
    )  # xx=0, all j, both slots
    zcol3 = bass.AP(zs[:].tensor, 0, [[0, 67], [1, 3 * 256]])
    nc.scalar.dma_start(
        out=bass.AP(img2[:].tensor, 65 * 256, [[IW * 256, 67], [1, 3 * 256]]),
        in_=zcol3,
    )  # xx=65..67

    # ---------------- Stage B2: gather indices (16-part layout) ------
    NCH = 4
    TPC = NT // NCH
    frees = TPC * NK * 8

    def emit_b2_chunk(ci):
        def st(name):
            return s6pool.tile([128, frees], F32, name=name, tag="st16")[0:16]

        o16 = s6pool.tile([128, 2 * frees], F32, name="o16", tag="o16")[0:16]
        src = bass.AP(
            off[:].tensor,
            ci * TPC * 128 * 18,
            [[18, 16], [128 * 18, TPC], [16 * 18, 8], [1, 18]],
        )
        nc.sync.dma_start(
            out=o16[:, :].rearrange("q (t g c) -> q t g c", g=8, c=18), in_=src
        )
        o16v = o16[:, :].rearrange("q (t g c) -> q t c g", g=8, c=18)

        by6 = s6pool.tile([128, frees], F32, name="by6", tag="b16")[0:16]
        bx6 = s6pool.tile([128, frees], F32, name="bx6", tag="b16")[0:16]
        cslc = slice(ci * frees, (ci + 1) * frees)
        nc.sync.dma_start(out=by6[:], in_=by16_d[:, cslc])
        nc.sync.dma_start(out=bx6[:], in_=bx16_d[:, cslc])

        py6 = st("py6")
        px6 = st("px6")
        nc.vector.tensor_add(
            py6[:].rearrange("q (t k g) -> q t k g", k=NK, g=8),
            o16v[:, :, 0:NK, :],
            by6[:].rearrange("q (t k g) -> q t k g", k=NK, g=8),
        )
        nc.vector.tensor_add(
            px6[:].rearrange("q (t k g) -> q t k g", k=NK, g=8),
            o16v[:, :, NK : 2 * NK, :],
            bx6[:].rearrange("q (t k g) -> q t k g", k=NK, g=8),
        )
        y0f6 = st("y0f6")
        x0f6 = st("x0f6")
        nc.vector.tensor_scalar(y0f6[:], py6[:], -0.5, MAGIC, alu.add, alu.add)
        nc.vector.tensor_scalar(y0f6[:], y0f6[:], MAGIC, None, alu.subtract)
        nc.vector.tensor_scalar(x0f6[:], px6[:], -0.5, MAGIC, alu.add, alu.add)
        nc.vector.tensor_scalar(x0f6[:], x0f6[:], MAGIC, None, alu.subtract)
        ye6 = st("ye6")
        x06 = st("x06")
        nc.vector.tensor_scalar(ye6[:], y0f6[:], -1.0, CLIP_MAX, alu.max, alu.min)
        nc.vector.tensor_scalar(x06[:], x0f6[:], 0.0, CLIP_MAX, alu.max, alu.min)
        i0 = st("i0")
        nc.vector.tensor_scalar(i0[:], ye6[:], float(IW), float(IW), alu.mult, alu.add)
        nc.vector.tensor_add(i0[:], i0[:], x06[:])
        cs = slice(ci * TPC * 72, (ci + 1) * TPC * 72)
        nc.vector.tensor_copy(idx[0:16, cs], i0[:])
        dslc = idxd[ci * 16 * 576 : (ci + 1) * 16 * 576]
        nc.scalar.dma_start(out=dslc.rearrange("(q n) -> q n", n=576), in_=idx[0:16, cs])
        rep = bass.AP(dslc.tensor, dslc.offset, [[0, 7], [576, 16], [1, 576]])
        nc.scalar.dma_start(out=idx[16:128, cs], in_=rep)

    emit_b2_chunk(0)
    nc.sync.dma_start(out=byp[:], in_=byp_d[:])
    nc.sync.dma_start(out=bxp[:], in_=bxp_d[:])
    nc.sync.dma_start(out=mask[:], in_=mask_d[:])

    # ---------------- Stage B1: pix-major corner weights -------------
    offv = off[:].rearrange("(t p) c -> p t c", p=128)
    offp = spool.tile([128, NT, 2 * NK], F32, name="offp")
    nc.sync.dma_start(out=offp[:], in_=offv)
    modv = mod[:].rearrange("(t p) c -> p t c", p=128)
    modp = spool.tile([128, NT, NK], F32, name="modp")
    nc.sync.dma_start(out=modp[:], in_=modv)
    modf = modp[:].rearrange("p t k -> p (t k)")

    def wt(name):
        return wpool.tile([128, NTK], F32, name=name, tag="wta")

    py = wt("py")
    px = wt("px")
    nc.vector.tensor_add(
        py[:], offp[:, :, 0:NK], byp[:].rearrange("p (t k) -> p t k", k=NK)
    )
    nc.vector.tensor_add(
        px[:], offp[:, :, NK : 2 * NK], bxp[:].rearrange("p (t k) -> p t k", k=NK)
    )

    y0f = wt("y0f")
    x0f = wt("x0f")
    nc.vector.tensor_scalar(y0f[:], py[:], -0.5, MAGIC, alu.add, alu.add)
    nc.vector.tensor_scalar(y0f[:], y0f[:], MAGIC, None, alu.subtract)
    nc.vector.tensor_scalar(x0f[:], px[:], -0.5, MAGIC, alu.add, alu.add)
    nc.vector.tensor_scalar(x0f[:], x0f[:], MAGIC, None, alu.subtract)

    y0 = wt("y0")
    x0 = wt("x0")
    x1c = wt("x1c")
    nc.vector.tensor_scalar(y0[:], y0f[:], 0.0, CLIP_MAX, alu.max, alu.min)
    nc.vector.tensor_scalar(x0[:], x0f[:], 0.0, CLIP_MAX, alu.max, alu.min)
    nc.vector.tensor_scalar(x1c[:], x0f[:], 1.0, 0.0, alu.add, alu.max)
    nc.vector.tensor_scalar(x1c[:], x1c[:], CLIP_MAX, None, alu.min)

    pyc = wt("pyc")
    pxc = wt("pxc")
    nc.vector.tensor_scalar(pyc[:], py[:], 0.0, CLIP_MAX, alu.max, alu.min)
    nc.vector.tensor_scalar(pxc[:], px[:], 0.0, CLIP_MAX, alu.max, alu.min)
    ly = wt("ly")
    lx = wt("lx")
    sx = wt("sx")
    nc.vector.tensor_sub(ly[:], pyc[:], y0[:])
    nc.vector.tensor_sub(lx[:], pxc[:], x0[:])
    nc.vector.tensor_sub(sx[:], x1c[:], x0[:])

    # corner weights; pairing per reference (transposed lx/ly roles):
    # (y0,x0):oly*olx  (y1,x0):oly*lx  (y0,x1):ly*olx  (y1,x1):ly*lx
    oly = wt("oly")
    olx = wt("olx")
    nc.vector.tensor_scalar(oly[:], ly[:], -1.0, 1.0, alu.mult, alu.add)
    nc.vector.tensor_scalar(olx[:], lx[:], -1.0, 1.0, alu.mult, alu.add)
    am = wt("am")
    bm = wt("bm")
    nc.vector.tensor_mul(am[:], olx[:], modf)
    nc.vector.tensor_mul(bm[:], lx[:], modf)
    a0 = wt("a0")
    b0 = wt("b0")
    a1 = wt("a1")
    b1 = wt("b1")
    nc.vector.tensor_mul(a0[:], oly[:], am[:])
    nc.vector.tensor_mul(b0[:], ly[:], am[:])
    nc.vector.tensor_mul(a1[:], oly[:], bm[:])
    nc.vector.tensor_mul(b1[:], ly[:], bm[:])

    osx = wt("osx")
    nc.vector.tensor_scalar(osx[:], sx[:], -1.0, 1.0, alu.mult, alu.add)

    # w4 slots (r0x0, r1x0, r0x1, r1x1); x1 slots gated by sx
    w4 = spool.tile([128, NTK, 4], F32, name="w4")
    tmp = wt("tmpw")
    nc.vector.tensor_mul(tmp[:], osx[:], b0[:])
    nc.vector.tensor_add(w4[:, :, 0], tmp[:], a0[:])
    nc.vector.tensor_mul(tmp[:], osx[:], b1[:])
    nc.vector.tensor_add(w4[:, :, 1], tmp[:], a1[:])
    nc.vector.tensor_mul(w4[:, :, 2], sx[:], b0[:])
    nc.vector.tensor_mul(w4[:, :, 3], sx[:], b1[:])
    # duplicate each weight twice (uint32 AND packing)
    w4df = w4dup[:].rearrange("p (a b) -> p a b", b=2)
    w4s = w4[:].rearrange("p a c -> p (a c)")
    w4bc = bass.AP(w4s.tensor, w4s.offset, [list(w4s.ap[0]), list(w4s.ap[1]), [0, 2]])
    nc.vector.tensor_copy(w4df, w4bc)

    for _ci in range(1, NCH):
        emit_b2_chunk(_ci)

    # ---------------- Stage C: gather + weighted bilinear transpose ---
    img_gsrc = bass.AP(img2[:].tensor, 0, [[256, 67 * IW - 1], [1, 512]])
    w32all = w4dup[:].bitcast(U32)  # [128, NTK*4]
    m32 = mask[:].bitcast(U32)      # [128, 64]
    m32b = bass.AP(m32.tensor, m32.offset,
                   [list(m32.ap[0]), [0, NK * 4], list(m32.ap[1])])

    def q(i):
        return (i // 6) * 6 + (i % 3) * 2 + ((i % 6) // 3)

    def tile_compute(t, g):
        # dg[p, (k,cr), :] = mask row * w4[p, (t,k,cr)] via uint32 AND
        dg = dgpool.tile([128, NK * 4, 128], BF16, name="dg")
        dg32 = dg[:].bitcast(U32)
        w32 = bass.AP(
            w32all.tensor,
            w32all.offset + t * NK * 4,
            [list(w32all.ap[0]), [1, NK * 4], [0, 64]],
        )
        nc.vector.tensor_tensor(dg32, m32b, w32, alu.bitwise_and)

        for k in range(NK):
            pf = ppool.tile([128, 128], F32, name="pfeats")
            for cr in range(4):
                nc.tensor.matmul(
                    pf[:],
                    g[:, k, cr * 128 : (cr + 1) * 128],
                    dg[:, k * 4 + cr, :],
                    start=(cr == 0),
                    stop=(cr == 3),
                )
            s, j = k // 3, k % 3
            i0_ = s * 64 + 2 * t
            q0, q1 = q(i0_), q(i0_ + 1)
            base = j * 12288 + q0 * 64
            dstap = bass.AP(
                feats.tensor,
                feats.offset + base,
                [list(feats.ap[0]), [(q1 - q0) * 64, 2], [1, 64]],
            )
            nc.scalar.activation(dstap, pf[:], act_copy)

        # conv for output tile T once its three feats tiles are written
        for T in range(NT):
            if max((3 * T + u) % NT for u in range(3)) != t:
                continue
            po = p2pool.tile([128, F], F32, name="pout")
            n = 0
            for r in range(3):
                for j in range(3):
                    base = j * 12288 + (T * 6 + r * 2) * 64
                    lhsT = feats[:, base : base + 128]
                    nc.tensor.matmul(
                        po[:],
                        lhsT,
                        kmat[:, (r * 3 + j) * F : (r * 3 + j + 1) * F],
                        start=(n == 0),
                        stop=(n == 8),
                    )
                    n += 1
            ot = opool.tile([128, F], F32, name="ot")
            nc.scalar.activation(ot[:], po[:], act_copy)
            nc.sync.dma_start(out=out[T * 128 : (T + 1) * 128, :], in_=ot[:])

    # conv kernel f32 load (sync ring) + DVE cast to bf16: keeps the Pool
    # queue free for gathers
    ckstage = dgpool.tile([128, NK * 4, 128], BF16, name="dg")
    ckf32 = ckstage[:].rearrange("p a b -> p (a b)").bitcast(F32)  # [128, 2304]
    nc.sync.dma_start(out=ckf32, in_=ck[:].rearrange("k c f -> c k f"))
    nc.vector.tensor_copy(kmat[:], ckf32)

    TPG = 1  # sample tiles per gather call
    for call in range(NT // TPG):
        gg = gpool.tile([128, TPG * NK, 512], FP8, name="gt")
        nc.gpsimd.dma_gather(
            gg[:],
            img_gsrc,
            idx[:, call * TPG * 72 : (call + 1) * TPG * 72],
            num_idxs=TPG * NK * 128,
            num_idxs_reg=TPG * NK * 128,
            elem_size=512,
            elem_step=256,
            single_packet=False,
            queue_num=(0, 2, 1, 3)[call % NQ],
        )
        for ti in range(TPG):
            t = call * TPG + ti
            tile_compute(t, gg[:, ti * NK : (ti + 1) * NK, :])
    ctx.close()


_CACHED_NC = None


def _get_nc():
    global _CACHED_NC
    if _CACHED_NC is None:
        _CACHED_NC = build_nc()
    return _CACHED_NC


def kernel(x, offset, modulation, conv_kernel, conv_bias):
    from concourse.bass_utils import run_bass_kernel_spmd

    B = x.shape[0]
    consts = _host_consts()
    ck9 = np.ascontiguousarray(conv_kernel.reshape(NK, C, F), dtype=np.float32)
    in_maps = []
    for b in range(B):
        in_maps.append(
            {
                "x": np.ascontiguousarray(x[b].reshape(NPIX, C), np.float32),
                "off": np.ascontiguousarray(
                    offset[b].reshape(NPIX, 2 * NK), np.float32
                ),
                "mod": np.ascontiguousarray(
                    modulation[b].reshape(NPIX, NK), np.float32
                ),
                "ck": ck9,
                "byp": consts["byp"],
                "bxp": consts["bxp"],
                "by16": consts["by16"],
                "bx16": consts["bx16"],
                "mask": consts["mask"],
            }
        )
    nc = _get_nc()
    res = run_bass_kernel_spmd(
        nc,
        in_maps,
        core_ids=list(range(B)),
        trace=bool(int(os.environ.get("KERNEL_TRACE", "0"))),
    )
    outs = [res.results[b]["out"].reshape(H, W, F) for b in range(B)]
    result = np.stack(outs, axis=0) + conv_bias[None, None, None, :]
    if getattr(res, "exec_time_ns", None):
        kernel.last_exec_time_ns = res.exec_time_ns
    return result.astype(np.float32)

